# revision 46
# baseline (speedup 1.0000x reference)
"""AttentionWithBiasAndExtraOutput on 8 TRN2 NeuronCores.

Sharding: core c -> (batch b = c//2, head-half g = c%2).  Each core computes
8 heads of one batch: projections with its 512-row slice of Wq/Wk/Wv
(SCALE folded into Wq on host), qk (extra output), softmax(qk+bias), attn@v,
and its half of the output projection.  The out_proj partial sums from the
two head-halves of a batch are added on the host (plus bo) - no device
collectives.

All TensorE matmuls run in float32r (full-rate fp32).  The walrus verifier
requires every f32r matmul operand to be produced by an engine op that
rounds to f32r, so DMA-loaded operands get a one-time rounding copy and all
intermediate PE operands are materialized as f32r tiles.

Device layouts (per core):
  qT/kT/vT   (DIM, N)  - host-pre-transposed so the contraction dim (C) is on
                         SBUF partitions for the projection matmuls.
  wqT/wkT/wvT (DIM, 512), woT (512, DIM) - pre-sliced / pre-transposed weights.
  bias       (8, N, S) - this core's (batch, head-range) slice.
Outputs: qk (8, N, S), y (N, DIM) partial of out_proj.
"""

import sys

if "/opt/trn_rl_repo" not in sys.path:
    sys.path.insert(0, "/opt/trn_rl_repo")

from contextlib import ExitStack

import numpy as np

import concourse.bass as bass
import concourse.mybir as mybir
from concourse import bacc
import concourse.tile as tile
from concourse.masks import make_identity

B, N, S, DIM, H, D = 4, 1024, 1024, 1024, 16, 64
HL = 8            # heads per core
DH = HL * D       # 512: per-core slice of the head-concat dim
SCALE = D ** -0.5
NCORES = 8

F32 = mybir.dt.float32
F32R = mybir.dt.float32r
BF16 = mybir.dt.bfloat16
EXP = mybir.ActivationFunctionType.Exp


def _r(ap):
    return ap.bitcast(F32R)


def build_nc() -> bass.Bass:
    nc = bacc.Bacc()

    qT = nc.declare_dram_parameter("qT", [DIM, N], BF16, isOutput=False)
    kT = nc.declare_dram_parameter("kT", [DIM, S], BF16, isOutput=False)
    vT = nc.declare_dram_parameter("vT", [DIM, S], BF16, isOutput=False)
    wqT = nc.declare_dram_parameter("wqT", [DIM, DH], BF16, isOutput=False)
    wkT = nc.declare_dram_parameter("wkT", [DIM, DH], BF16, isOutput=False)
    wvT = nc.declare_dram_parameter("wvT", [DIM, DH], BF16, isOutput=False)
    woT = nc.declare_dram_parameter("woT", [DH, DIM], F32, isOutput=False)
    biasd = nc.declare_dram_parameter("bias", [HL, N, S], BF16, isOutput=False)
    qkd = nc.declare_dram_parameter("qk", [HL, N, S], F32, isOutput=True)
    yd = nc.declare_dram_parameter("y", [N, DIM], F32, isOutput=True)

    # Round-robin elementwise work between ScalarE and VectorE.
    _cp_state = [0]

    def cp(out, in_):
        _cp_state[0] ^= 1
        if _cp_state[0]:
            nc.scalar.copy(out, in_)
        else:
            nc.vector.tensor_copy(out, in_)

    with tile.TileContext(nc) as tc, ExitStack() as ctx:
        resid = ctx.enter_context(tc.tile_pool(name="resid", bufs=1))

        # Persistent per-core tensors (live for the whole kernel).
        qhT_sb = resid.tile([128, 4, N], F32R)   # [p, ci, n]: qhT[ci*128+p, n]
        khT_sb = resid.tile([128, 4, S], F32R)   # [p, ci, s]
        vh_sb = resid.tile([128, 8, DH], BF16)   # [p, si, d]: vh[si*128+p, d]
        wo_sb = resid.tile([128, 4, DIM], F32R)  # [p, ci, j]: woT[ci*128+p, j]
        ident_raw = resid.tile([128, 128], F32)
        ident_bf = resid.tile([128, 128], BF16)
        make_identity(nc, ident_raw)
        nc.vector.tensor_copy(ident_bf, ident_raw)

        # HAM warm-up: ~5us of dummy matmuls while the first DMAs land, so
        # the PE clock is at 2.4 GHz when real work starts.
        with tc.tile_pool(name="warm", bufs=1, space="PSUM") as warm_pool:
            wps = warm_pool.tile([128, 512], F32)
            for i in range(12):
                for j in range(4):
                    nc.tensor.matmul(
                        wps[:, j * 128:(j + 1) * 128],
                        lhsT=ident_bf, rhs=ident_bf,
                        start=True, stop=True,
                    )

        # ---------------- Phase 1: load + round weights, projections --------
        with tc.tile_pool(name="raw", bufs=4) as raw_pool, \
             tc.tile_pool(name="xin", bufs=2) as xin_pool, \
             tc.tile_pool(name="wt", bufs=2) as wt_pool, \
             tc.tile_pool(name="ps512", bufs=4, space="PSUM") as ps512:

            for ci in range(4):
                rw = raw_pool.tile([128, 1024], F32, tag="raw")
                nc.sync.dma_start(out=rw, in_=woT[ci * 128:(ci + 1) * 128, :])
                cp(wo_sb[:, ci, :], rw)

            def load_rounded(xTd, wd):
                # bf16 inputs stream straight from DRAM - no rounding pass.
                xin = xin_pool.tile([128, 8, 1024], BF16)  # [p, kc, col]
                w = wt_pool.tile([128, 8, DH], BF16)       # [p, kc, d]
                for kc in range(8):
                    nc.sync.dma_start(out=xin[:, kc, :], in_=xTd[kc * 128:(kc + 1) * 128, :])
                    nc.sync.dma_start(out=w[:, kc, :], in_=wd[kc * 128:(kc + 1) * 128, :])
                return xin, w

            qxin, qw = load_rounded(qT, wqT)
            kxin, kw = load_rounded(kT, wkT)
            # q/k projections in kc-outer waves: 4 concurrent PSUM groups per
            # wave, so the first matmul only needs the first kc chunk of the
            # DMA stream instead of all 8.
            for dst, xin, w in ((qhT_sb, qxin, qw), (khT_sb, kxin, kw)):
                for wave in range(2):
                    pss = [ps512.tile([128, 512], F32, name="ps") for _ in range(4)]
                    for kc in range(8):
                        for g in range(4):
                            ci, nh = (wave * 4 + g) // 2, (wave * 4 + g) % 2
                            nc.tensor.matmul(
                                pss[g],
                                lhsT=w[:, kc, ci * 128:(ci + 1) * 128],
                                rhs=xin[:, kc, nh * 512:(nh + 1) * 512],
                                start=(kc == 0), stop=(kc == 7),
                            )
                    for g in range(4):
                        ci, nh = (wave * 4 + g) // 2, (wave * 4 + g) % 2
                        cp(dst[:, ci, nh * 512:(nh + 1) * 512], pss[g])
            vxin, vw = load_rounded(vT, wvT)
            # vh natural: rows = s (8 chunks), cols = dh (512)
            for si in range(8):
                ps = ps512.tile([128, 512], F32)
                for kc in range(8):
                    nc.tensor.matmul(
                        ps,
                        lhsT=vxin[:, kc, si * 128:(si + 1) * 128],
                        rhs=vw[:, kc, :],
                        start=(kc == 0), stop=(kc == 7),
                    )
                cp(vh_sb[:, si, :], ps)

        # ---------------- Phase 2: attention per head ----------------
        # Head-skewed pipeline: the qk/softmax stage of head h is emitted
        # before the transpose/attn@v tail of head h-1, so TensorE always has
        # qk matmuls available while ScalarE/VectorE finish the softmax.
        with tc.tile_pool(name="e", bufs=2) as e_pool, \
             tc.tile_pool(name="eT", bufs=2) as eT_pool, \
             tc.tile_pool(name="x", bufs=1) as x_pool, \
             tc.tile_pool(name="qko", bufs=3) as qko_pool, \
             tc.tile_pool(name="bias", bufs=3) as bias_pool, \
             tc.tile_pool(name="stat", bufs=4) as stat_pool, \
             tc.tile_pool(name="qt", bufs=3, space="PSUM") as qt_pool, \
             tc.tile_pool(name="sm", bufs=2, space="PSUM") as sm_pool:

            # xT[c-chunk partitions, n]: x[n, ci*128+p] - written directly by
            # the oT-form attn@v (head h owns partition rows (h%2)*64..+64 of
            # chunk h//2), consumed by the out projection.
            xT = x_pool.tile([128, 4, N], F32R)
            state = {}

            def qk_stage(h):
                ci, po = h // 2, (h % 2) * 64
                sums = stat_pool.tile([128, 8], F32, tag="sums")
                e = e_pool.tile([128, 8, S], BF16)   # [p, nc, s]
                state[h] = (sums, e)
                # bias[h]/qk[h] viewed as [p, j, s] with n = j*128+p: one DMA
                # carries two n-chunks (1 MB-class transfers).
                bias_v = biasd[h].rearrange("(j p) s -> p j s", p=128)
                qk_v = qkd[h].rearrange("(j p) s -> p j s", p=128)
                for ncp in range(4):
                    qk_sb = qko_pool.tile([128, 2, S], F32, tag="qk_sb")
                    bias_sb = bias_pool.tile([128, 2, S], BF16)
                    nc.sync.dma_start(
                        out=bias_sb, in_=bias_v[:, 2 * ncp:2 * ncp + 2, :])
                    for half in range(2):
                        ncb = 2 * ncp + half
                        qt = qt_pool.tile([128, 1024], F32)  # 2 PSUM banks
                        for sh in range(2):
                            nc.tensor.matmul(
                                qt[:, sh * 512:(sh + 1) * 512],
                                lhsT=qhT_sb[po:po + 64, ci, ncb * 128:(ncb + 1) * 128],
                                rhs=khT_sb[po:po + 64, ci, sh * 512:(sh + 1) * 512],
                                start=True, stop=True,
                            )
                        nc.vector.tensor_copy(qk_sb[:, half, :], qt)
                        # accumulate bias on top of qk in PSUM (after the raw
                        # qk has been copied out for the extra output)
                        for sh in range(2):
                            nc.tensor.matmul(
                                qt[:, sh * 512:(sh + 1) * 512],
                                lhsT=ident_bf,
                                rhs=bias_sb[:, half, sh * 512:(sh + 1) * 512],
                                start=False, stop=True, skip_group_check=True,
                            )
                        nc.scalar.activation(
                            state[h][1][:, ncb, :], qt, EXP,
                            accum_out=sums[:, ncb:ncb + 1])
                    nc.sync.dma_start(
                        out=qk_v[:, 2 * ncp:2 * ncp + 2, :], in_=qk_sb)

            def tail_stage(h):
                ci, po = h // 2, (h % 2) * 64
                sums, e = state.pop(h)
                rinv = stat_pool.tile([128, 8], F32, tag="rinv")
                nc.vector.reciprocal(rinv, sums)
                # normalize attn rows in place (bf16 tensor_scalar, 4x mode)
                for ncb in range(8):
                    nc.vector.tensor_scalar_mul(
                        e[:, ncb, :], e[:, ncb, :], rinv[:, ncb:ncb + 1])

                # transpose e -> eT  ([p, sc, n]: e[n, sc*128+p])
                eT = eT_pool.tile([128, 8, S], BF16)
                for sc in range(8):
                    trp = sm_pool.tile([128, 1024], BF16, tag="sm")
                    for nb in range(8):
                        nc.tensor.transpose(
                            trp[:, nb * 128:(nb + 1) * 128],
                            e[:, nb, sc * 128:(sc + 1) * 128],
                            ident_bf,
                        )
                    cp(eT[:, sc, :], trp)

                # oT = vh_h.T @ attn.T accumulated over s-chunks; rows land
                # directly in this head's partition slice of xT.
                for nh in range(2):
                    op = sm_pool.tile([64, 512], F32, tag="sm")
                    for sc in range(8):
                        nc.tensor.matmul(
                            op,
                            lhsT=vh_sb[:, sc, h * 64:(h + 1) * 64],
                            rhs=eT[:, sc, nh * 512:(nh + 1) * 512],
                            start=(sc == 0), stop=(sc == 7),
                        )
                    cp(xT[po:po + 64, ci, nh * 512:(nh + 1) * 512], op)

            for h in range(HL):
                qk_stage(h)
                if h >= 1:
                    tail_stage(h - 1)
            tail_stage(HL - 1)

            # ---------------- Phase 3: out projection ----------------
            for ncb in range(8):
                y_sb = qko_pool.tile([128, 2, 512], F32, tag="qk_sb")
                qt = qt_pool.tile([128, 1024], F32)
                for jh in range(2):
                    for ci in range(4):
                        nc.tensor.matmul(
                            qt[:, jh * 512:(jh + 1) * 512],
                            lhsT=xT[:, ci, ncb * 128:(ncb + 1) * 128],
                            rhs=wo_sb[:, ci, jh * 512:(jh + 1) * 512],
                            start=(ci == 0), stop=(ci == 3),
                        )
                cp(y_sb.rearrange("p a b -> p (a b)"), qt)
                nc.sync.dma_start(
                    out=yd[ncb * 128:(ncb + 1) * 128, :],
                    in_=y_sb.rearrange("p a b -> p (a b)"))

    nc.finalize()
    return nc


_NC = None


def _get_nc() -> bass.Bass:
    global _NC
    if _NC is None:
        _NC = build_nc()
    return _NC


def make_in_maps(q, k, v, bias, Wq, Wk, Wv, Wo):
    import ml_dtypes

    bias_bf = bias.astype(ml_dtypes.bfloat16)
    in_maps = []
    for c in range(NCORES):
        b, g = divmod(c, 2)
        rs = slice(g * DH, (g + 1) * DH)
        bf = ml_dtypes.bfloat16
        in_maps.append({
            "qT": np.ascontiguousarray(q[b].T.astype(bf)),
            "kT": np.ascontiguousarray(k[b].T.astype(bf)),
            "vT": np.ascontiguousarray(v[b].T.astype(bf)),
            "wqT": np.ascontiguousarray((SCALE * Wq[rs]).T.astype(bf)),
            "wkT": np.ascontiguousarray(Wk[rs].T.astype(bf)),
            "wvT": np.ascontiguousarray(Wv[rs].T.astype(bf)),
            "woT": np.ascontiguousarray(Wo[:, rs].T),
            "bias": np.ascontiguousarray(bias_bf[:, g * HL:(g + 1) * HL][b]),
        })
    return in_maps


def assemble(results, bo):
    qk = np.empty((B, H, N, S), np.float32)
    out = np.empty((B, N, DIM), np.float32)
    for c in range(NCORES):
        b, g = divmod(c, 2)
        qk[b, g * HL:(g + 1) * HL] = results[c]["qk"]
    for b in range(B):
        out[b] = results[2 * b]["y"] + results[2 * b + 1]["y"] + bo[None, :]
    return out, qk


def run(inputs, trace=False, trace_cores=None):
    """Returns ((out, qk), BassKernelResults)."""
    from concourse.bass_utils import run_bass_kernel_spmd

    arr = {k2: np.asarray(v2, np.float32) for k2, v2 in inputs.items()}
    in_maps = make_in_maps(arr["q"], arr["k"], arr["v"], arr["bias"],
                           arr["Wq"], arr["Wk"], arr["Wv"], arr["Wo"])
    br = run_bass_kernel_spmd(
        _get_nc(), in_maps, list(range(NCORES)),
        trace=trace, trace_cores=trace_cores)
    return assemble(br.results, arr["bo"]), br


def kernel(**inputs):
    (out, qk), _ = run(inputs, trace=False)
    return out, qk


# revision 47
# speedup vs baseline: 1.0424x; 1.0424x over previous
"""AttentionWithBiasAndExtraOutput on 8 TRN2 NeuronCores.

Sharding: core c -> (batch b = c//2, head-half g = c%2).  Each core computes
8 heads of one batch: projections with its 512-row slice of Wq/Wk/Wv
(SCALE folded into Wq on host), qk (extra output), softmax(qk+bias), attn@v,
and its half of the output projection.  The out_proj partial sums from the
two head-halves of a batch are added on the host (plus bo) - no device
collectives.

All TensorE matmuls run in float32r (full-rate fp32).  The walrus verifier
requires every f32r matmul operand to be produced by an engine op that
rounds to f32r, so DMA-loaded operands get a one-time rounding copy and all
intermediate PE operands are materialized as f32r tiles.

Device layouts (per core):
  qT/kT/vT   (DIM, N)  - host-pre-transposed so the contraction dim (C) is on
                         SBUF partitions for the projection matmuls.
  wqT/wkT/wvT (DIM, 512), woT (512, DIM) - pre-sliced / pre-transposed weights.
  bias       (8, N, S) - this core's (batch, head-range) slice.
Outputs: qk (8, N, S), y (N, DIM) partial of out_proj.
"""

import sys

if "/opt/trn_rl_repo" not in sys.path:
    sys.path.insert(0, "/opt/trn_rl_repo")

from contextlib import ExitStack

import numpy as np

import concourse.bass as bass
import concourse.mybir as mybir
from concourse import bacc
import concourse.tile as tile
from concourse.masks import make_identity

B, N, S, DIM, H, D = 4, 1024, 1024, 1024, 16, 64
HL = 8            # heads per core
DH = HL * D       # 512: per-core slice of the head-concat dim
SCALE = D ** -0.5
NCORES = 8

F32 = mybir.dt.float32
F32R = mybir.dt.float32r
BF16 = mybir.dt.bfloat16
EXP = mybir.ActivationFunctionType.Exp


def _r(ap):
    return ap.bitcast(F32R)


def build_nc() -> bass.Bass:
    nc = bacc.Bacc()

    qT = nc.declare_dram_parameter("qT", [DIM, N], BF16, isOutput=False)
    kT = nc.declare_dram_parameter("kT", [DIM, S], BF16, isOutput=False)
    vT = nc.declare_dram_parameter("vT", [DIM, S], BF16, isOutput=False)
    wqT = nc.declare_dram_parameter("wqT", [DIM, DH], BF16, isOutput=False)
    wkT = nc.declare_dram_parameter("wkT", [DIM, DH], BF16, isOutput=False)
    wvT = nc.declare_dram_parameter("wvT", [DIM, DH], BF16, isOutput=False)
    woT = nc.declare_dram_parameter("woT", [DH, DIM], F32, isOutput=False)
    biasd = nc.declare_dram_parameter("bias", [HL, N, S], BF16, isOutput=False)
    qkd = nc.declare_dram_parameter("qk", [HL, N, S], F32, isOutput=True)
    yd = nc.declare_dram_parameter("y", [N, DIM], F32, isOutput=True)

    # Round-robin elementwise work between ScalarE and VectorE.
    _cp_state = [0]

    def cp(out, in_):
        _cp_state[0] ^= 1
        if _cp_state[0]:
            nc.scalar.copy(out, in_)
        else:
            nc.vector.tensor_copy(out, in_)

    with tile.TileContext(nc) as tc, ExitStack() as ctx:
        resid = ctx.enter_context(tc.tile_pool(name="resid", bufs=1))

        # Persistent per-core tensors (live for the whole kernel).
        qhT_sb = resid.tile([128, 4, N], F32R)   # [p, ci, n]: qhT[ci*128+p, n]
        khT_sb = resid.tile([128, 4, S], F32R)   # [p, ci, s]
        vh_sb = resid.tile([128, 8, DH], BF16)   # [p, si, d]: vh[si*128+p, d]
        wo_sb = resid.tile([128, 4, DIM], F32R)  # [p, ci, j]: woT[ci*128+p, j]
        ident_raw = resid.tile([128, 128], F32)
        ident_bf = resid.tile([128, 128], BF16)
        make_identity(nc, ident_raw)
        nc.vector.tensor_copy(ident_bf, ident_raw)

        # HAM warm-up: dummy matmuls on a zeroed tile while the first DMAs
        # land, so the PE clock is at 2.4 GHz when real work starts.  DVE
        # memset is ready instantly (no gpsimd library load on the path).
        wtile = resid.tile([128, 128], BF16)
        nc.vector.memset(wtile, 0.0)
        with tc.tile_pool(name="warm", bufs=1, space="PSUM") as warm_pool:
            wps = warm_pool.tile([128, 512], F32)
            for i in range(12):
                for j in range(4):
                    nc.tensor.matmul(
                        wps[:, j * 128:(j + 1) * 128],
                        lhsT=wtile, rhs=wtile,
                        start=True, stop=True,
                    )

        # ---------------- Phase 1: load + round weights, projections --------
        with tc.tile_pool(name="raw", bufs=4) as raw_pool, \
             tc.tile_pool(name="xin", bufs=2) as xin_pool, \
             tc.tile_pool(name="wt", bufs=2) as wt_pool, \
             tc.tile_pool(name="ps512", bufs=4, space="PSUM") as ps512:

            def load_rounded(xTd, wd):
                # bf16 inputs stream straight from DRAM - no rounding pass.
                xin = xin_pool.tile([128, 8, 1024], BF16)  # [p, kc, col]
                w = wt_pool.tile([128, 8, DH], BF16)       # [p, kc, d]
                for kc in range(8):
                    nc.sync.dma_start(out=xin[:, kc, :], in_=xTd[kc * 128:(kc + 1) * 128, :])
                    nc.sync.dma_start(out=w[:, kc, :], in_=wd[kc * 128:(kc + 1) * 128, :])
                return xin, w

            qxin, qw = load_rounded(qT, wqT)
            kxin, kw = load_rounded(kT, wkT)
            # q/k projections in kc-outer waves: 4 concurrent PSUM groups per
            # wave, so the first matmul only needs the first kc chunk of the
            # DMA stream instead of all 8.
            for dst, xin, w in ((qhT_sb, qxin, qw), (khT_sb, kxin, kw)):
                for wave in range(2):
                    pss = [ps512.tile([128, 512], F32, name="ps") for _ in range(4)]
                    for kc in range(8):
                        for g in range(4):
                            ci, nh = (wave * 4 + g) // 2, (wave * 4 + g) % 2
                            nc.tensor.matmul(
                                pss[g],
                                lhsT=w[:, kc, ci * 128:(ci + 1) * 128],
                                rhs=xin[:, kc, nh * 512:(nh + 1) * 512],
                                start=(kc == 0), stop=(kc == 7),
                            )
                    for g in range(4):
                        ci, nh = (wave * 4 + g) // 2, (wave * 4 + g) % 2
                        cp(dst[:, ci, nh * 512:(nh + 1) * 512], pss[g])
            for ci in range(4):
                rw = raw_pool.tile([128, 1024], F32, tag="raw")
                nc.sync.dma_start(out=rw, in_=woT[ci * 128:(ci + 1) * 128, :])
                cp(wo_sb[:, ci, :], rw)
            vxin, vw = load_rounded(vT, wvT)
            # vh natural: rows = s (8 chunks), cols = dh (512)
            for si in range(8):
                ps = ps512.tile([128, 512], F32)
                for kc in range(8):
                    nc.tensor.matmul(
                        ps,
                        lhsT=vxin[:, kc, si * 128:(si + 1) * 128],
                        rhs=vw[:, kc, :],
                        start=(kc == 0), stop=(kc == 7),
                    )
                cp(vh_sb[:, si, :], ps)

        # ---------------- Phase 2: attention per head ----------------
        # Head-skewed pipeline: the qk/softmax stage of head h is emitted
        # before the transpose/attn@v tail of head h-1, so TensorE always has
        # qk matmuls available while ScalarE/VectorE finish the softmax.
        with tc.tile_pool(name="e", bufs=2) as e_pool, \
             tc.tile_pool(name="eT", bufs=2) as eT_pool, \
             tc.tile_pool(name="x", bufs=1) as x_pool, \
             tc.tile_pool(name="qko", bufs=3) as qko_pool, \
             tc.tile_pool(name="bias", bufs=3) as bias_pool, \
             tc.tile_pool(name="stat", bufs=4) as stat_pool, \
             tc.tile_pool(name="qt", bufs=3, space="PSUM") as qt_pool, \
             tc.tile_pool(name="sm", bufs=2, space="PSUM") as sm_pool:

            # xT[c-chunk partitions, n]: x[n, ci*128+p] - written directly by
            # the oT-form attn@v (head h owns partition rows (h%2)*64..+64 of
            # chunk h//2), consumed by the out projection.
            xT = x_pool.tile([128, 4, N], F32R)
            state = {}

            def qk_stage(h):
                ci, po = h // 2, (h % 2) * 64
                sums = stat_pool.tile([128, 8], F32, tag="sums")
                e = e_pool.tile([128, 8, S], BF16)   # [p, nc, s]
                state[h] = (sums, e)
                # bias[h]/qk[h] viewed as [p, j, s] with n = j*128+p: one DMA
                # carries two n-chunks (1 MB-class transfers).
                bias_v = biasd[h].rearrange("(j p) s -> p j s", p=128)
                qk_v = qkd[h].rearrange("(j p) s -> p j s", p=128)
                for ncp in range(4):
                    qk_sb = qko_pool.tile([128, 2, S], F32, tag="qk_sb")
                    bias_sb = bias_pool.tile([128, 2, S], BF16)
                    nc.sync.dma_start(
                        out=bias_sb, in_=bias_v[:, 2 * ncp:2 * ncp + 2, :])
                    for half in range(2):
                        ncb = 2 * ncp + half
                        qt = qt_pool.tile([128, 1024], F32)  # 2 PSUM banks
                        for sh in range(2):
                            nc.tensor.matmul(
                                qt[:, sh * 512:(sh + 1) * 512],
                                lhsT=qhT_sb[po:po + 64, ci, ncb * 128:(ncb + 1) * 128],
                                rhs=khT_sb[po:po + 64, ci, sh * 512:(sh + 1) * 512],
                                start=True, stop=True,
                            )
                        nc.vector.tensor_copy(qk_sb[:, half, :], qt)
                        # accumulate bias on top of qk in PSUM (after the raw
                        # qk has been copied out for the extra output)
                        for sh in range(2):
                            nc.tensor.matmul(
                                qt[:, sh * 512:(sh + 1) * 512],
                                lhsT=ident_bf,
                                rhs=bias_sb[:, half, sh * 512:(sh + 1) * 512],
                                start=False, stop=True, skip_group_check=True,
                            )
                        nc.scalar.activation(
                            state[h][1][:, ncb, :], qt, EXP,
                            accum_out=sums[:, ncb:ncb + 1])
                    nc.sync.dma_start(
                        out=qk_v[:, 2 * ncp:2 * ncp + 2, :], in_=qk_sb)

            def tail_stage(h):
                ci, po = h // 2, (h % 2) * 64
                sums, e = state.pop(h)
                rinv = stat_pool.tile([128, 8], F32, tag="rinv")
                nc.vector.reciprocal(rinv, sums)
                # normalize attn rows in place (bf16 tensor_scalar, 4x mode)
                for ncb in range(8):
                    nc.vector.tensor_scalar_mul(
                        e[:, ncb, :], e[:, ncb, :], rinv[:, ncb:ncb + 1])

                # transpose e -> eT  ([p, sc, n]: e[n, sc*128+p])
                eT = eT_pool.tile([128, 8, S], BF16)
                for sc in range(8):
                    trp = sm_pool.tile([128, 1024], BF16, tag="sm")
                    for nb in range(8):
                        nc.tensor.transpose(
                            trp[:, nb * 128:(nb + 1) * 128],
                            e[:, nb, sc * 128:(sc + 1) * 128],
                            ident_bf,
                        )
                    cp(eT[:, sc, :], trp)

                # oT = vh_h.T @ attn.T accumulated over s-chunks; rows land
                # directly in this head's partition slice of xT.
                for nh in range(2):
                    op = sm_pool.tile([64, 512], F32, tag="sm")
                    for sc in range(8):
                        nc.tensor.matmul(
                            op,
                            lhsT=vh_sb[:, sc, h * 64:(h + 1) * 64],
                            rhs=eT[:, sc, nh * 512:(nh + 1) * 512],
                            start=(sc == 0), stop=(sc == 7),
                        )
                    cp(xT[po:po + 64, ci, nh * 512:(nh + 1) * 512], op)

            for h in range(HL):
                qk_stage(h)
                if h >= 1:
                    tail_stage(h - 1)
            tail_stage(HL - 1)

            # ---------------- Phase 3: out projection ----------------
            for ncb in range(8):
                y_sb = qko_pool.tile([128, 2, 512], F32, tag="qk_sb")
                qt = qt_pool.tile([128, 1024], F32)
                for jh in range(2):
                    for ci in range(4):
                        nc.tensor.matmul(
                            qt[:, jh * 512:(jh + 1) * 512],
                            lhsT=xT[:, ci, ncb * 128:(ncb + 1) * 128],
                            rhs=wo_sb[:, ci, jh * 512:(jh + 1) * 512],
                            start=(ci == 0), stop=(ci == 3),
                        )
                cp(y_sb.rearrange("p a b -> p (a b)"), qt)
                nc.sync.dma_start(
                    out=yd[ncb * 128:(ncb + 1) * 128, :],
                    in_=y_sb.rearrange("p a b -> p (a b)"))

    nc.finalize()
    return nc


_NC = None


def _get_nc() -> bass.Bass:
    global _NC
    if _NC is None:
        _NC = build_nc()
    return _NC


def make_in_maps(q, k, v, bias, Wq, Wk, Wv, Wo):
    import ml_dtypes

    bias_bf = bias.astype(ml_dtypes.bfloat16)
    in_maps = []
    for c in range(NCORES):
        b, g = divmod(c, 2)
        rs = slice(g * DH, (g + 1) * DH)
        bf = ml_dtypes.bfloat16
        in_maps.append({
            "qT": np.ascontiguousarray(q[b].T.astype(bf)),
            "kT": np.ascontiguousarray(k[b].T.astype(bf)),
            "vT": np.ascontiguousarray(v[b].T.astype(bf)),
            "wqT": np.ascontiguousarray((SCALE * Wq[rs]).T.astype(bf)),
            "wkT": np.ascontiguousarray(Wk[rs].T.astype(bf)),
            "wvT": np.ascontiguousarray(Wv[rs].T.astype(bf)),
            "woT": np.ascontiguousarray(Wo[:, rs].T),
            "bias": np.ascontiguousarray(bias_bf[:, g * HL:(g + 1) * HL][b]),
        })
    return in_maps


def assemble(results, bo):
    qk = np.empty((B, H, N, S), np.float32)
    out = np.empty((B, N, DIM), np.float32)
    for c in range(NCORES):
        b, g = divmod(c, 2)
        qk[b, g * HL:(g + 1) * HL] = results[c]["qk"]
    for b in range(B):
        out[b] = results[2 * b]["y"] + results[2 * b + 1]["y"] + bo[None, :]
    return out, qk


def run(inputs, trace=False, trace_cores=None):
    """Returns ((out, qk), BassKernelResults)."""
    from concourse.bass_utils import run_bass_kernel_spmd

    arr = {k2: np.asarray(v2, np.float32) for k2, v2 in inputs.items()}
    in_maps = make_in_maps(arr["q"], arr["k"], arr["v"], arr["bias"],
                           arr["Wq"], arr["Wk"], arr["Wv"], arr["Wo"])
    br = run_bass_kernel_spmd(
        _get_nc(), in_maps, list(range(NCORES)),
        trace=trace, trace_cores=trace_cores)
    return assemble(br.results, arr["bo"]), br


def kernel(**inputs):
    (out, qk), _ = run(inputs, trace=False)
    return out, qk
